# revision 5
# baseline (speedup 1.0000x reference)
"""Trainium2 Bass kernel for nn_Attention_56169582297517.

ref:  q = primary @ W.T + b            [N,L]
      k = secondary @ W.T + b          [M,L]
      s = relu(q @ k.T)                [N,M]
      s = s / max(||s||_row, 1e-12)
      out = s @ secondary              [N,E]

N=M=8192, E=512, L=128.  Sharding: primary rows split across 8 cores
(1024 rows each); secondary/W/b replicated; each core computes its row
slice independently (row-wise L2 norm is local to N).

Per-core plan (normalization deferred to the very end):
  out_row = (relu(q k^T) @ S)_row / max(norm_row, eps)

Scores are computed TRANSPOSED (m on partitions, n on free) so the
context matmul can contract m on partitions against natural-layout
secondary chunks.

The context matmul (66% of PE work at bf16) runs in fp8e4 with
perf_mode=DoubleRow: two m-chunks packed per matmul ([K=128, 2, *] APs
on both operands), doubling contraction throughput.  Scores are relu'd
+ cast to fp8 by the DVE (ACT cannot write fp8 - hangs the exec unit);
max score ~112 < 240 so no scaling is needed.  Secondary is resident
as fp8 m-chunk pairs [P, 32, 2, E].

The row norm is ALSO computed on the PE: a second DoubleRow matmul per
(pair, n-block) accumulates the gram diag blocks st8^T @ st8 into one
PSUM bank; the diagonal (= sum_m s^2 over the same fp8 values the ctx
matmul uses) is extracted at group end by scalar_tensor_tensor against
the identity with accum_out.  This removes the elementwise
squares/accumulate traffic entirely.  Simulated rel err vs the fp32
reference is ~1.05e-2 (gate 2e-2).

All e-contraction transposes (W, primary, secondary) run on the XBAR
DMA transpose engine (dma_start_transpose, bf16), not the PE; this
also frees the PSUM bank the gram accumulator needs.  The secondary
pipeline is 3 superchunks deep (load -> bf16-cast+transpose -> kproj)
so the in-order PE stream never waits on a transpose.  q/k projections
and the scores matmul run in bf16.

PSUM budget (8 banks): proj(1) + scores(2x2) + ctx(4) + gram(1) = 8.
"""

import sys
import types

import numpy as np
from contextlib import ExitStack

import concourse.bass as bass
import concourse.bacc as bacc
import concourse.mybir as mybir
import concourse.tile as tile
from concourse.bass_utils import run_bass_kernel_spmd
from concourse.masks import make_identity


def _install_ntff_shim():
    """Some images lack antenv.axon_hooks; synthesize it so
    run_bass_kernel_spmd(trace=True) (or BASS_TRACE=1) can't crash on the
    import, and wire the NTFF profile hook when the axon .so supports it."""
    if "antenv.axon_hooks" in sys.modules:
        return
    try:
        import antenv
        import antenv.axon_hooks  # noqa: F401
        return  # real module exists
    except ImportError:
        pass
    try:
        mod = types.ModuleType("antenv.axon_hooks")
        mod._hook = None
        mod.set_axon_ntff_profile_hook = lambda h: setattr(mod, "_hook", h)
        mod.get_axon_ntff_profile_hook = lambda: mod._hook
        sys.modules["antenv.axon_hooks"] = mod
        antenv.axon_hooks = mod
        try:
            from trn_agent_boot.trn_boot import _ntff_profile_via_ctypes

            hook = _ntff_profile_via_ctypes("/opt/axon/libaxon_pjrt.so")
            if hook is not None:
                mod.set_axon_ntff_profile_hook(hook)
        except Exception:
            pass
    except Exception:
        pass


_install_ntff_shim()

N_CORES = 8
N, M, E, L = 8192, 8192, 512, 128
NLOC = N // N_CORES          # 1024 primary rows per core
P = 128
EC = E // P                  # 4 e-chunks of 128
M_CHUNKS = M // P            # 64 m-chunks of 128
M_PAIRS = M_CHUNKS // 2      # 32 fp8 DoubleRow pairs
SC = 4                       # m-chunks per load superchunk (512 rows)
N_SUPER = M_CHUNKS // SC     # 16
PPS = SC // 2                # pairs per superchunk (2)
NG = 512                     # n-group width (psum free dim)
N_GROUPS = NLOC // NG        # 2
NB = NG // P                 # 4 n-blocks of 128 per group
EPS = 1e-12

F32 = mybir.dt.float32
BF16 = mybir.dt.bfloat16
FP8 = mybir.dt.float8e4
AF = mybir.ActivationFunctionType
ALU = mybir.AluOpType
DR = mybir.MatmulPerfMode.DoubleRow


def _emit(nc: bass.Bass):
    prim = nc.dram_tensor("primary", [NLOC, E], F32, kind="ExternalInput")
    sec = nc.dram_tensor("secondary", [M, E], F32, kind="ExternalInput")
    w_d = nc.dram_tensor("W", [L, E], F32, kind="ExternalInput")
    b_d = nc.dram_tensor("b", [L], F32, kind="ExternalInput")
    out_d = nc.dram_tensor("out", [NLOC, E], F32, kind="ExternalOutput")

    with tile.TileContext(nc) as tc, ExitStack() as ctx:
        consts = ctx.enter_context(tc.tile_pool(name="consts", bufs=1))
        big = ctx.enter_context(tc.tile_pool(name="big", bufs=1))
        stage = ctx.enter_context(tc.tile_pool(name="stage", bufs=2))
        work = ctx.enter_context(tc.tile_pool(name="work", bufs=3))
        psum = ctx.enter_context(tc.tile_pool(name="psum", bufs=1, space="PSUM"))

        # ---------------- constants ----------------
        ident = consts.tile([P, P], F32)
        make_identity(nc, ident)
        b_sb = consts.tile([P, 1], F32)
        with nc.allow_non_contiguous_dma(reason="128x4B bias load, one-off"):
            nc.sync.dma_start(b_sb, b_d[:].rearrange("(p o) -> p o", o=1))
        w_sb = consts.tile([P, E], F32)
        nc.sync.dma_start(w_sb, w_d[:])

        # secondary superchunk loads issued first: they are the long pole
        s_f32s = {}

        def emit_load(sc):
            s_f32 = stage.tile([P, SC, E], F32, tag="sstage", name="s_f32", bufs=4)
            if sc == 0:
                # first superchunk: per-j loads so the pipeline's first data
                # lands as early as possible
                for j in range(SC):
                    nc.sync.dma_start(s_f32[:, j, :], sec[(sc * SC + j) * P:(sc * SC + j + 1) * P, :])
            else:
                nc.sync.dma_start(
                    s_f32,
                    sec[sc * SC * P:(sc + 1) * SC * P, :].rearrange("(j p) e -> p j e", p=P),
                )
            s_f32s[sc] = s_f32

        emit_load(0)
        emit_load(1)
        emit_load(2)

        # W^T via XBAR dma transpose: wt[e_in, ec, l]
        w_bf = consts.tile([P, E], BF16)
        nc.scalar.copy(w_bf, w_sb)
        wt = consts.tile([P, EC, P], BF16)
        nc.sync.dma_start_transpose(wt, w_bf)

        # ---------------- qT = W @ P_loc^T + b  -> [l, n]  (bf16) ----------------
        qt = big.tile([P, NLOC], BF16)
        for h in range(NLOC // NG):
            pq = psum.tile([P, NG], F32, tag="proj", name="pq")
            for nb4 in range(NB):
                pc = stage.tile([P, E], F32, tag="pchunk", name="pc")
                nc.sync.dma_start(pc, prim[(h * NB + nb4) * P:(h * NB + nb4 + 1) * P, :])
                pc_bf = stage.tile([P, E], BF16, tag="pchunk_bf", name="pc_bf")
                nc.vector.tensor_copy(pc_bf, pc)
                pt_sb = stage.tile([P, EC, P], BF16, tag="pt", name="pt_sb")
                nc.sync.dma_start_transpose(pt_sb, pc_bf)
                for e in range(EC):
                    nc.tensor.matmul(
                        pq[:, nb4 * P:(nb4 + 1) * P],
                        lhsT=wt[:, e, :],
                        rhs=pt_sb[:, e, :],
                        start=(e == 0),
                        stop=(e == EC - 1),
                    )
            nc.scalar.activation(qt[:, h * NG:(h + 1) * NG], pq, AF.Identity, bias=b_sb)

        # ------------- secondary: fp8 pairs, bf16 transpose, kT projection -------------
        s8 = big.tile([P, M_PAIRS, 2, E], FP8)     # [m_in, pair, j, e]
        kt = big.tile([P, M], BF16)                # [l, m]
        st_sbs = {}

        def emit_xpose(sc):
            # bf16 cast (ACT) + XBAR dma transpose -> st_sb[e_in, j, ec, m]
            s_bf = stage.tile([P, SC, E], BF16, tag="sbf", name="s_bf")
            nc.scalar.copy(s_bf, s_f32s[sc])
            st_sb = stage.tile([P, SC, EC, P], BF16, tag="st", name="st_sb")
            nc.sync.dma_start_transpose(st_sb, s_bf)
            st_sbs[sc] = st_sb

        def emit_kproj(sc):
            st_sb = st_sbs.pop(sc)
            pk = psum.tile([P, SC * P], F32, tag="proj", name="pk")
            for e in range(EC):
                nc.tensor.matmul(
                    pk,
                    lhsT=wt[:, e, :],
                    rhs=st_sb[:, :, e, :],
                    start=(e == 0),
                    stop=(e == EC - 1),
                )
            nc.scalar.activation(kt[:, sc * SC * P:(sc + 1) * SC * P], pk, AF.Identity, bias=b_sb)

        def emit_s8cast(sc):
            s_f32 = s_f32s.pop(sc)
            for i in range(PPS):
                nc.vector.tensor_copy(s8[:, sc * PPS + i, :, :], s_f32[:, 2 * i:2 * i + 2, :])

        # ---------------- main loop: scores^T, gram norms, context ----------------
        def emit_scores_pair(g, mp):
            tiles = []
            for j in range(2):
                sc_ps = psum.tile([P, NG], F32, tag="scores", name="sc_ps", bufs=2)
                nc.tensor.matmul(
                    sc_ps,
                    lhsT=kt[:, (2 * mp + j) * P:(2 * mp + j + 1) * P],
                    rhs=qt[:, g * NG:(g + 1) * NG],
                    start=True,
                    stop=True,
                )
                tiles.append(sc_ps)
            return tiles

        def emit_group_prologue(g):
            ctx_ps = [
                psum.tile([P, E], F32, tag=f"ctx{jb}", name=f"ctx{jb}") for jb in range(NB)
            ]
            gram_ps = psum.tile([P, NB * P], F32, tag="gram", name="gram_ps")
            return {"ctx_ps": ctx_ps, "gram_ps": gram_ps,
                    "sc": emit_scores_pair(g, 0)}

        def emit_pair(g, st, mp):
            st8 = work.tile([P, 2, NG], FP8, tag="st8", name="st8", bufs=3)
            # relu + fp8 cast on DVE (ACT cannot write fp8)
            nc.vector.tensor_scalar_max(st8[:, 0, :], st["sc"][0], 0.0)
            nc.vector.tensor_scalar_max(st8[:, 1, :], st["sc"][1], 0.0)
            # next pair's scores issued ahead so the in-order PE stream has
            # work while the DVE produces this pair's fp8 tile
            if mp + 1 < M_PAIRS:
                st["sc"] = emit_scores_pair(g, mp + 1)
            for jb in range(NB):
                lhsT = st8[:, :, jb * P:(jb + 1) * P]
                nc.tensor.matmul(
                    st["ctx_ps"][jb],
                    lhsT=lhsT,
                    rhs=s8[:, mp, :, :],
                    start=(mp == 0),
                    stop=(mp == M_PAIRS - 1),
                    perf_mode=DR,
                )
                # row-norm accumulation: gram diag block, same stationary tile
                nc.tensor.matmul(
                    st["gram_ps"][:, jb * P:(jb + 1) * P],
                    lhsT=lhsT,
                    rhs=lhsT,
                    start=(mp == 0),
                    stop=(mp == M_PAIRS - 1),
                    perf_mode=DR,
                )

        def emit_group_finalize(g, st):
            # ------- out = ctx / max(sqrt(diag(gram)), eps) -------
            n2 = work.tile([P, NB], F32, tag="n2", name="n2", bufs=1)
            for jb in range(NB):
                scratch = work.tile([P, P], F32, tag="scr", name="scratch", bufs=2)
                nc.vector.scalar_tensor_tensor(
                    scratch, st["gram_ps"][:, jb * P:(jb + 1) * P], 1.0, ident,
                    ALU.mult, ALU.mult, accum_out=n2[:, jb:jb + 1],
                )
            nrm = work.tile([P, NB], F32, tag="nrm", name="nrm", bufs=1)
            nc.scalar.activation(nrm, n2, AF.Sqrt)
            nrm_c = work.tile([P, NB], F32, tag="nrmc", name="nrm_c", bufs=1)
            nc.vector.tensor_scalar_max(nrm_c, nrm, EPS)
            recip = work.tile([P, NB], F32, tag="recip", name="recip", bufs=1)
            nc.vector.reciprocal(recip, nrm_c)
            for jb in range(NB):
                o_sb = work.tile([P, E], F32, tag="osb", name="o_sb", bufs=2)
                nc.scalar.activation(o_sb, st["ctx_ps"][jb], AF.Copy, scale=recip[:, jb:jb + 1])
                r0 = g * NG + jb * P
                nc.sync.dma_start(out_d[r0:r0 + P, :], o_sb)

        # Phase-0 production interleaved with group 0's consumption, three
        # superchunks deep (load sc+3 / transpose sc+2 / kproj sc+1) so each
        # stage has a full iteration of slack before its consumer.
        emit_xpose(0)
        emit_xpose(1)
        emit_kproj(0)
        emit_s8cast(0)
        st0 = emit_group_prologue(0)
        for sc in range(N_SUPER):
            if sc + 3 < N_SUPER:
                emit_load(sc + 3)
            if sc + 2 < N_SUPER:
                emit_xpose(sc + 2)
            if sc + 1 < N_SUPER:
                emit_kproj(sc + 1)
            for mp in range(sc * PPS, (sc + 1) * PPS):
                emit_pair(0, st0, mp)
            if sc + 1 < N_SUPER:
                emit_s8cast(sc + 1)
        emit_group_finalize(0, st0)

        st1 = emit_group_prologue(1)
        for mp in range(M_PAIRS):
            emit_pair(1, st1, mp)
        emit_group_finalize(1, st1)

    return nc


_NC_CACHE = None


def _get_nc():
    global _NC_CACHE
    if _NC_CACHE is None:
        nc = bacc.Bacc("TRN2", target_bir_lowering=False, debug=False)
        _emit(nc)
        nc.finalize()
        _NC_CACHE = nc
    return _NC_CACHE


def run_sharded(inputs, **kw):
    nc = _get_nc()
    prim = np.ascontiguousarray(np.asarray(inputs["primary"], dtype=np.float32))
    sec = np.ascontiguousarray(np.asarray(inputs["secondary"], dtype=np.float32))
    w = np.ascontiguousarray(np.asarray(inputs["W"], dtype=np.float32))
    b = np.ascontiguousarray(np.asarray(inputs["b"], dtype=np.float32))
    assert prim.shape == (N, E) and sec.shape == (M, E)
    assert w.shape == (L, E) and b.shape == (L,)
    in_maps = [
        {
            "primary": prim[i * NLOC:(i + 1) * NLOC],
            "secondary": sec,
            "W": w,
            "b": b,
        }
        for i in range(N_CORES)
    ]
    res = run_bass_kernel_spmd(nc, in_maps, list(range(N_CORES)), **kw)
    out = np.concatenate([res.results[i]["out"] for i in range(N_CORES)], axis=0)
    return out, res


def kernel(**inputs) -> np.ndarray:
    out, _ = run_sharded(inputs)
    return out
